# revision 1
# baseline (speedup 1.0000x reference)
"""LoRA attention (B=32, N=577, C=768, H=12, d=64, R=64) on 8 TRN2 cores.

Data-parallel over batch: 4 images per core, weights replicated, no
collectives. All activations are kept channel-major ([channel, token]) so
the whole chain — QKV, S_T = K_h^T Q_h, exp, O_T = V_aug^T E, proj — runs
on the PE with no on-device transposes. Softmax is computed without max
subtraction (scores are ~N(0, 0.3), exp is safe in fp32); the denominator
comes from a ones-augmented V stationary (M=65), is reciprocated on DVE,
broadcast across partitions with a K=1 PE matmul, and multiplied in at
PSUM-eviction time. Matmuls run as float32r (full rate at N>=256; token
splits are 320/260, 3-column overlap). The nk axis is tiled 4x128+65; the ones column of
V_aug is loaded from a host constant so no engine has to write f32r
consts.
"""

import contextlib

import numpy as np

import concourse.bacc as bacc
import concourse.mybir as mybir
import concourse.tile as tile
from concourse.bass_utils import run_bass_kernel_spmd

NCORES = 8
B, N, C = 32, 577, 768
H, D, R = 12, 64, 64
BC = B // NCORES    # batches per core
KT = C // 128       # 6 k-tiles over channels
NT = 5              # nk tiles: 4*128 + 65
TS = [128, 128, 128, 128, 65]          # nk tile sizes
TO = [0, 128, 256, 384, 512]           # nk tile offsets
W = [320, 260]                         # nq split widths (f32r: >=256, even)
OFF = [0, 317]                         # 3-column overlap keeps both even
NJ = 2
SCALE = D ** -0.5

F32 = mybir.dt.float32
F32R = mybir.dt.float32r
EXP = mybir.ActivationFunctionType.Exp
BF16 = mybir.dt.bfloat16


def build_program(repeat=0):
    nc = bacc.Bacc("TRN2", target_bir_lowering=False, debug=False,
                   enable_asserts=True, num_devices=NCORES)

    xt_d = nc.dram_tensor("xT", [BC, C, N], F32R, kind="ExternalInput").ap()
    wqk_d = nc.dram_tensor("w_qk", [128, 12, KT, 128], F32R, kind="ExternalInput").ap()
    wv_d = nc.dram_tensor("w_v", [128, KT, C], F32R, kind="ExternalInput").ap()
    akv_d = nc.dram_tensor("a_kv", [128, KT, 128], F32R, kind="ExternalInput").ap()
    bk_d = nc.dram_tensor("b_k", [64, KT, 128], F32R, kind="ExternalInput").ap()
    bv_d = nc.dram_tensor("b_v", [64, C], F32R, kind="ExternalInput").ap()
    wp_d = nc.dram_tensor("w_p", [128, KT, KT, 128], F32R, kind="ExternalInput").ap()
    pb_d = nc.dram_tensor("p_b", [128, KT], F32, kind="ExternalInput").ap()
    onescol_d = nc.dram_tensor("ones_col", [128, NT, H, 1], F32R, kind="ExternalInput").ap()
    onesrow_d = nc.dram_tensor("ones_row", [1, 64], F32R, kind="ExternalInput").ap()
    yt_d = nc.dram_tensor("yT", [BC, C, N], F32, kind="ExternalOutput").ap()

    with tile.TileContext(nc) as tc:
        with (
            tc.tile_pool(name="const", bufs=1) as cpool,
            tc.tile_pool(name="xin", bufs=2) as xpool,
            tc.tile_pool(name="qk", bufs=1) as qkpool,
            tc.tile_pool(name="vau", bufs=1) as vpool,
            tc.tile_pool(name="exp", bufs=6) as epool,
            tc.tile_pool(name="onorm", bufs=1) as opool,
            tc.tile_pool(name="small", bufs=3) as smpool,
            tc.tile_pool(name="yout", bufs=2) as ypool,
            tc.tile_pool(name="ps", bufs=3, space="PSUM") as pspool,
            tc.tile_pool(name="pso", bufs=3, space="PSUM") as psopool,
            tc.tile_pool(name="psb", bufs=2, space="PSUM") as psbpool,
        ):
            # --- resident weights (first-use order, sliced) ---
            akv = cpool.tile([128, KT * 128], F32R)
            nc.sync.dma_start(out=akv[:, :], in_=akv_d.rearrange("p k c -> p (k c)"))
            wqk = cpool.tile([128, 12 * KT * 128], F32R)
            for m in range(12):
                nc.sync.dma_start(out=wqk[:, m * KT * 128:(m + 1) * KT * 128],
                                  in_=wqk_d[:, m].rearrange("p k c -> p (k c)"))
            bkv = cpool.tile([128, C], F32R)
            nc.sync.dma_start(out=bkv[0:64, :], in_=bk_d.rearrange("p k c -> p (k c)"))
            nc.sync.dma_start(out=bkv[64:128, :], in_=bv_d[:, :])
            wv = cpool.tile([128, KT * C], F32R)
            for k in range(KT):
                nc.sync.dma_start(out=wv[:, k * C:(k + 1) * C], in_=wv_d[:, k])
            wp = cpool.tile([128, KT * KT * 128], F32R)
            for m in range(KT):
                nc.sync.dma_start(out=wp[:, m * KT * 128:(m + 1) * KT * 128],
                                  in_=wp_d[:, m].rearrange("p k c -> p (k c)"))
            pb = cpool.tile([128, KT], F32)
            nc.sync.dma_start(out=pb[:, :], in_=pb_d[:, :])
            onesrow = cpool.tile([1, 64], F32R)
            nc.sync.dma_start(out=onesrow[:, :], in_=onesrow_d[:, :])

            loop_cm = tc.For_i(0, repeat, 1) if repeat else contextlib.nullcontext()
            with loop_cm:
                for b in range(BC):
                    # --- x^T for this batch: [128, KT, N] ---
                    xt = xpool.tile([128, KT * N], F32R, tag="xt")
                    for k in range(KT):
                        nc.sync.dma_start(
                            out=xt[:, k * N:(k + 1) * N],
                            in_=xt_d[b, k * 128:(k + 1) * 128, :],
                        )

                    # --- LoRA down: u = A_kv @ x -> [128(r_k|r_v), N] ---
                    u = xpool.tile([128, N], F32R, tag="u")
                    for j in range(NJ):
                        ps = pspool.tile([128, 320], F32, tag="mm")
                        for k in range(KT):
                            nc.tensor.matmul(
                                ps[:, :W[j]],
                                lhsT=akv[:, k * 128:(k + 1) * 128],
                                rhs=xt[:, k * N + OFF[j]: k * N + OFF[j] + W[j]],
                                start=(k == 0), stop=(k == KT - 1),
                            )
                        nc.vector.tensor_copy(
                            u[:, OFF[j]: OFF[j] + W[j]], ps[:, :W[j]])

                    # --- q, k projections (+ fused LoRA delta on k) ---
                    qk = qkpool.tile([128, 12 * N], F32R, tag="qk")
                    for m in range(12):
                        for j in range(NJ):
                            ps = pspool.tile([128, 320], F32, tag="mm")
                            for k in range(KT):
                                nc.tensor.matmul(
                                    ps[:, :W[j]],
                                    lhsT=wqk[:, (m * KT + k) * 128:(m * KT + k + 1) * 128],
                                    rhs=xt[:, k * N + OFF[j]: k * N + OFF[j] + W[j]],
                                    start=(k == 0),
                                    stop=(k == KT - 1 and m < 6),
                                )
                            if m >= 6:  # k-head LoRA: += B_k^T-tile @ u_k
                                nc.tensor.matmul(
                                    ps[:, :W[j]],
                                    lhsT=bkv[0:64, (m - 6) * 128:(m - 5) * 128],
                                    rhs=u[0:64, OFF[j]: OFF[j] + W[j]],
                                    start=False, stop=True,
                                )
                            col = m * N + OFF[j]
                            nc.vector.tensor_copy(qk[:, col: col + W[j]], ps[:, :W[j]])

                    # --- V token-major, ones-augmented: [128, nt, 12, 65] ---
                    vaug = vpool.tile([128, NT * H * 65], F32R, tag="vaug")
                    for nt in range(NT):
                        t = TS[nt]
                        for j in range(2):  # oc splits of 384
                            ps = pspool.tile([128, 384], F32, tag="mm")
                            for k in range(KT):
                                nc.tensor.matmul(
                                    ps[:t, :],
                                    lhsT=xt[:, k * N + TO[nt]: k * N + TO[nt] + t],
                                    rhs=wv[:, k * C + j * 384: k * C + j * 384 + 384],
                                    start=(k == 0), stop=False,
                                )
                            nc.tensor.matmul(  # v-head LoRA: += u_v-tile @ B_v^T
                                ps[:t, :],
                                lhsT=u[64:128, TO[nt]: TO[nt] + t],
                                rhs=bkv[64:128, j * 384: j * 384 + 384],
                                start=False, stop=True,
                            )
                            dst = vaug[:t, nt * H * 65 + j * 6 * 65:
                                       nt * H * 65 + (j + 1) * 6 * 65]
                            nc.vector.tensor_copy(
                                dst.rearrange("p (h c) -> p h c", h=6)[:, :, 0:64],
                                ps[:t, :].rearrange("p (h c) -> p h c", h=6),
                            )
                    nc.sync.dma_start(
                        out=vaug[:, :].rearrange(
                            "p (t h c) -> p t h c", t=NT, h=H)[:, :, :, 64:65],
                        in_=onescol_d[:, :, :, :],
                    )

                    # --- attention per head (pair p, half h) ---
                    onorm = opool.tile([128, KT * N], F32R, tag="onorm")
                    for p in range(6):
                        for h in range(2):
                            base = h * 64
                            hh = 2 * p + h
                            ops = [psopool.tile([65, 320], F32, tag="o", name=f"o{j}")
                                   for j in range(NJ)]
                            for nt in range(NT):
                                t = TS[nt]
                                et = epool.tile([128, N], F32R, tag="e")
                                for j in range(NJ):
                                    ps = pspool.tile([128, 320], F32, tag="mm")
                                    nc.tensor.matmul(
                                        ps[:t, :W[j]],
                                        lhsT=qk[base:base + 64,
                                                (6 + p) * N + TO[nt]:(6 + p) * N + TO[nt] + t],
                                        rhs=qk[base:base + 64, p * N + OFF[j]: p * N + OFF[j] + W[j]],
                                        start=True, stop=True,
                                    )
                                    nc.scalar.activation(
                                        et[:t, OFF[j]: OFF[j] + W[j]],
                                        ps[:t, :W[j]], EXP, scale=SCALE)
                                for j in range(NJ):
                                    nc.tensor.matmul(
                                        ops[j][:, :W[j]],
                                        lhsT=vaug[:t, nt * H * 65 + hh * 65:
                                                  nt * H * 65 + hh * 65 + 65],
                                        rhs=et[:t, OFF[j]: OFF[j] + W[j]],
                                        start=(nt == 0), stop=(nt == NT - 1),
                                    )
                            recip = smpool.tile([1, N], F32R, tag="recip")
                            with nc.allow_low_precision(reason="f32r recip for bcast"):
                                for j in range(NJ):
                                    nc.vector.reciprocal(
                                        recip[:, OFF[j]: OFF[j] + W[j]],
                                        ops[j][64:65, :W[j]])
                            bcast = smpool.tile([64, N], F32, tag="bcast")
                            for j in range(NJ):
                                psb = psbpool.tile([64, 320], F32, tag="ob", name="psb")
                                nc.tensor.matmul(
                                    psb[:, :W[j]], lhsT=onesrow[:, :],
                                    rhs=recip[:, OFF[j]: OFF[j] + W[j]],
                                    start=True, stop=True,
                                )
                                nc.vector.tensor_copy(
                                    bcast[:, OFF[j]: OFF[j] + W[j]], psb[:, :W[j]])
                            if h == 0:
                                for j in range(NJ):
                                    nc.vector.tensor_mul(
                                        onorm[0:64, p * N + OFF[j]: p * N + OFF[j] + W[j]],
                                        ops[j][0:64, :W[j]],
                                        bcast[:, OFF[j]: OFF[j] + W[j]],
                                    )
                            else:
                                stage = smpool.tile([64, N], F32R, tag="stage")
                                for j in range(NJ):
                                    nc.vector.tensor_mul(
                                        stage[:, OFF[j]: OFF[j] + W[j]],
                                        ops[j][0:64, :W[j]],
                                        bcast[:, OFF[j]: OFF[j] + W[j]],
                                    )
                                nc.sync.dma_start(
                                    out=onorm[64:128, p * N:(p + 1) * N], in_=stage[:, :])

                    # --- output projection + bias ---
                    for m in range(KT):
                        yt = ypool.tile([128, N], F32, tag="y")
                        for j in range(NJ):
                            ps = pspool.tile([128, 320], F32, tag="mm")
                            for k in range(KT):
                                nc.tensor.matmul(
                                    ps[:, :W[j]],
                                    lhsT=wp[:, (m * KT + k) * 128:(m * KT + k + 1) * 128],
                                    rhs=onorm[:, k * N + OFF[j]: k * N + OFF[j] + W[j]],
                                    start=(k == 0), stop=(k == KT - 1),
                                )
                            nc.vector.tensor_scalar_add(
                                yt[:, OFF[j]: OFF[j] + W[j]], ps[:, :W[j]],
                                pb[:, m: m + 1])
                        nc.sync.dma_start(
                            out=yt_d[b, m * 128:(m + 1) * 128, :], in_=yt[:, :])

    nc.compile()
    return nc


_NC = {}


def _get_nc(repeat=0):
    if repeat not in _NC:
        _NC[repeat] = build_program(repeat)
    return _NC[repeat]


def _ones_col():
    oc = np.zeros((128, NT, H, 1), np.float32)
    for t in range(NT):
        for p in range(128):
            if t * 128 + p < N:
                oc[p, t, :, 0] = 1.0
    return oc


def _prep_maps(x, qkv_w, proj_w, proj_b, lora_A_k, lora_B_k, lora_A_v, lora_B_v):
    f = np.float32
    x = np.asarray(x, f)
    qkv_w = np.asarray(qkv_w, f)
    proj_w = np.asarray(proj_w, f)
    proj_b = np.asarray(proj_b, f)
    A_kv = np.concatenate([np.asarray(lora_A_k, f), np.asarray(lora_A_v, f)], 0)
    B_k = np.asarray(lora_B_k, f)
    B_v = np.asarray(lora_B_v, f)

    shared = {
        "w_qk": np.ascontiguousarray(
            qkv_w[:2 * C].reshape(12, 128, KT, 128).transpose(3, 0, 2, 1)),
        "w_v": np.ascontiguousarray(
            qkv_w[2 * C:].T.reshape(KT, 128, C).transpose(1, 0, 2)),
        "a_kv": np.ascontiguousarray(
            A_kv.T.reshape(KT, 128, 128).transpose(1, 0, 2)),
        "b_k": np.ascontiguousarray(B_k.reshape(KT, 128, 64).transpose(2, 0, 1)),
        "b_v": np.ascontiguousarray(B_v.T),
        "w_p": np.ascontiguousarray(
            proj_w.reshape(KT, 128, KT, 128).transpose(3, 0, 2, 1)),
        "p_b": np.ascontiguousarray(proj_b.reshape(KT, 128).T),
        "ones_col": _ones_col(),
        "ones_row": np.ones((1, 64), np.float32),
    }
    in_maps = []
    for c in range(NCORES):
        xc = x[c * BC:(c + 1) * BC]                       # [BC, N, C]
        in_maps.append({"xT": np.ascontiguousarray(xc.transpose(0, 2, 1)), **shared})
    return in_maps


def kernel(x, task, qkv_w, proj_w, proj_b, lora_A_k, lora_B_k, lora_A_v,
           lora_B_v, _trace=False, _trace_kwargs=None, _repeat=0):
    nc = _get_nc(_repeat)
    in_maps = _prep_maps(x, qkv_w, proj_w, proj_b,
                         lora_A_k, lora_B_k, lora_A_v, lora_B_v)
    res = run_bass_kernel_spmd(nc, in_maps, list(range(NCORES)),
                               trace=_trace, **(_trace_kwargs or {}))
    out = np.empty((B, N, C), np.float32)
    for c in range(NCORES):
        yT = res.results[c]["yT"]                          # [BC, C, N]
        out[c * BC:(c + 1) * BC] = yT.transpose(0, 2, 1)
    if _trace:
        return out, res
    return out

